# revision 1
# baseline (speedup 1.0000x reference)
"""Trainium2 Bass kernel for a 4-layer dense MLP (H=8192), batch=1.

Tensor-parallel over 8 NeuronCores, structured to hide collective latency:

  - Layer 1 (10x8192) is replicated on every core, computed in 8 passes of
    1024 columns; each pass bounces through DRAM into the [128, 64] activation
    layout piecewise, so layer 2 can start contracting on early pieces while
    later pieces are still in flight (layer 2's contraction chunks are ordered
    piece-major via a host-side weight-row permutation).

  - Hidden layers 2-4 are column-sharded (core c owns 1024 columns) and each
    is computed as two 512-column halves A/B. Half A's AllGather fires at
    mid-layer and overlaps half B's compute; the NEXT layer's contraction
    chunks are ordered so chunks 0-31 touch only gathered-A data and 32-63
    only B (again via host-side row permutations), so the next layer starts
    as soon as AG-A lands while AG-B is still in flight. Exposed collective
    latency is nearly zero.

  - The output layer (8192x8) is row-sharded: no collective after layer 4;
    each core emits a partial [8] which the host sums.

  - A dummy AllGather fires at kernel start so the one-time ncfw rendezvous
    barrier overlaps layer-1 compute and weight prefetch.

Compute dtype is fp16 (PSUM accumulation is fp32); measured end-to-end error
vs the f32 reference is ~4e-4 max-rel. Weights stream as contiguous 1 MiB
DMAs into [128, 4096] SBUF tiles (8 contraction chunks x 512 columns each).
"""

import numpy as np

H = 8192
D = 10  # input layer size (4 + 6)
OUT = 8
NCORES = 8
SH = H // NCORES  # 1024 columns per core
HF = 512  # half-width
KC = 64  # contraction chunks of 128 rows
GC = 8  # chunks per DMA group (1 MiB per DMA at 512 cols)
G = KC // GC  # 8 groups per half
WBUFS = 16  # in-flight weight DMA buffers (16 MiB SBUF)

LAST_RESULTS = None
_CACHE = {}


def _perm_piece():
    """Layer-2 input layout: a_sb[p, k] = a1[(k//8)*1024 + p*8 + (k%8)].
    Returns rows[k, p] = global row index feeding chunk k, partition p."""
    k = np.arange(KC)[:, None]
    p = np.arange(128)[None, :]
    return (k // 8) * 1024 + p * 8 + (k % 8)


def _perm_ab():
    """Layer-3/4 input layout: chunks 0-31 hold the gathered A-halves
    (columns [0,512) of every rank), chunks 32-63 the B-halves.
    a_sb[p, k] = half_flat[p*32 + k%32] with half = k//32, and
    half_flat[i] = a_full[(i//512)*1024 + 512*half + i%512]."""
    k = np.arange(KC)[:, None]
    p = np.arange(128)[None, :]
    half = k // 32
    i = p * 32 + (k % 32)
    return (i // 512) * 1024 + 512 * half + (i % 512)


def _build_nc():
    import concourse.bacc as bacc
    import concourse.mybir as mybir
    import concourse.tile as tile

    f16 = mybir.dt.float16
    f32 = mybir.dt.float32
    SIG = mybir.ActivationFunctionType.Sigmoid
    RG = [list(range(NCORES))]

    nc = bacc.Bacc(
        "TRN2", target_bir_lowering=False, debug=False, num_devices=NCORES
    )

    x_d = nc.dram_tensor("x_cat", [D, 1], f16, kind="ExternalInput")
    win_d = nc.dram_tensor("w_in", [D, H], f16, kind="ExternalInput")
    whh_d = nc.dram_tensor("w_hh", [3, 2, G, 128, GC * HF], f16, kind="ExternalInput")
    wout_d = nc.dram_tensor("w_out", [128, 8 * OUT], f16, kind="ExternalInput")
    bias0_d = nc.dram_tensor("bias0", [1, H], f16, kind="ExternalInput")
    bias_d = nc.dram_tensor("bias", [1, 3 * SH], f16, kind="ExternalInput")
    out_d = nc.dram_tensor("out_partial", [1, OUT], f32, kind="ExternalOutput")

    with tile.TileContext(nc) as tc:
        with (
            tc.tile_pool(name="const", bufs=1) as cp,
            tc.tile_pool(name="wpool", bufs=WBUFS) as wp,
            tc.tile_pool(name="apool", bufs=2) as ap,
            tc.tile_pool(name="pspool", bufs=2, space="PSUM") as pp,
            tc.tile_pool(name="dpool", bufs=2, space="DRAM") as dp,
        ):
            one_sb = cp.tile([1, 1], f16)
            nc.gpsimd.memset(one_sb[:], 1.0)

            # Dummy collective: absorbs the one-time ncfw rendezvous barrier
            # concurrently with layer-1 compute + weight prefetch.
            warm_sb = cp.tile([1, 16], f16)
            nc.gpsimd.memset(warm_sb[:], 0.0)
            warm_in = dp.tile([1, 16], f16, tag="warmin")
            warm_out = dp.tile([8, 16], f16, tag="warmout")
            nc.gpsimd.dma_start(warm_in[:], warm_sb[:])
            nc.gpsimd.collective_compute(
                "AllGather",
                mybir.AluOpType.bypass,
                replica_groups=RG,
                ins=[warm_in.opt()],
                outs=[warm_out.opt()],
            )

            x_sb = cp.tile([D, 1], f16)
            nc.scalar.dma_start(x_sb[:], x_d[:])
            win_sb = cp.tile([D, H], f16)
            nc.scalar.dma_start(win_sb[:], win_d[:])
            bias0_sb = cp.tile([1, H], f16)
            nc.scalar.dma_start(bias0_sb[:], bias0_d[:])
            bias_sb = cp.tile([1, 3 * SH], f16)
            nc.scalar.dma_start(bias_sb[:], bias_d[:])
            wout_sb = cp.tile([128, 8 * OUT], f16)
            nc.scalar.dma_start(wout_sb[:], wout_d[:])

            # ---- Layer 1, replicated: 16 passes of 512 cols, piecewise
            # bounce into the [128, 64] piece-major layout for layer 2 ----
            act1_sb = cp.tile([1, H], f16)
            a1_d = dp.tile([16, HF], f16, tag="a1")
            a_sb = ap.tile([128, KC], f16, tag="a")
            for q in range(16):
                h, odd = q // 2, q % 2
                lo = q * HF
                ps1 = pp.tile([1, HF], f32, tag=f"ps{odd}")
                nc.tensor.matmul(
                    ps1[:],
                    x_sb[:],
                    win_sb[:, lo : lo + HF],
                    start=True,
                    stop=False,
                )
                nc.tensor.matmul(
                    ps1[:],
                    one_sb[:],
                    bias0_sb[:, lo : lo + HF],
                    start=False,
                    stop=True,
                )
                nc.scalar.activation(act1_sb[:, lo : lo + HF], ps1[:], SIG)
                nc.scalar.dma_start(
                    a1_d[q : q + 1, :], act1_sb[:, lo : lo + HF]
                )
                nc.scalar.dma_start(
                    a_sb[64 * odd : 64 * odd + 64, 8 * h : 8 * h + 8],
                    a1_d[q].rearrange("(p k) -> p k", p=64),
                )

            # ---- Hidden layers 2-4: two 512-col halves, AG-A at mid-layer ----
            act_half = [None, None]
            for li in range(3):
                a_next = (
                    ap.tile([128, KC], f16, tag="a", name="a_next")
                    if li < 2
                    else None
                )
                for hf in range(2):
                    ps = pp.tile([1, HF], f32, tag=f"ps{hf}")
                    for g in range(G):
                        wt = wp.tile([128, GC * HF], f16, tag="w")
                        nc.sync.dma_start(wt[:], whh_d[li, hf, g])
                        for c in range(GC):
                            k = g * GC + c
                            nc.tensor.matmul(
                                ps[:],
                                a_sb[:, k : k + 1],
                                wt[:, c * HF : (c + 1) * HF],
                                start=(k == 0),
                                stop=False,
                            )
                    nc.tensor.matmul(
                        ps[:],
                        one_sb[:],
                        bias_sb[:, li * SH + hf * HF : li * SH + hf * HF + HF],
                        start=False,
                        stop=True,
                    )
                    act_h = ap.tile([1, HF], f16, tag=f"act{hf}")
                    nc.scalar.activation(act_h[:], ps[:], SIG)
                    act_half[hf] = act_h
                    if li < 2:
                        cc_in = dp.tile([1, HF], f16, tag=f"ccin{hf}")
                        cc_out = dp.tile([128, 32], f16, tag=f"ccout{hf}")
                        nc.gpsimd.dma_start(cc_in[:], act_h[:])
                        nc.gpsimd.collective_compute(
                            "AllGather",
                            mybir.AluOpType.bypass,
                            replica_groups=RG,
                            ins=[cc_in.opt()],
                            outs=[cc_out.opt()],
                        )
                        nc.scalar.dma_start(
                            a_next[:, 32 * hf : 32 * hf + 32], cc_out[:]
                        )
                if li < 2:
                    a_sb = a_next

            # ---- Output layer: row-sharded, partial [8] per core ----
            sc = dp.tile([1, SH], f16, tag="sc")
            nc.scalar.dma_start(sc[:, 0:HF], act_half[0][:])
            nc.scalar.dma_start(sc[:, HF:SH], act_half[1][:])
            a2_sb = ap.tile([128, 8], f16, tag="a2")
            nc.scalar.dma_start(
                a2_sb[:], sc.rearrange("one (p k) -> (one p) k", p=128)
            )
            pso = pp.tile([1, OUT], f32, tag="psO", bufs=1)
            for k in range(8):
                nc.tensor.matmul(
                    pso[:],
                    a2_sb[:, k : k + 1],
                    wout_sb[:, k * OUT : (k + 1) * OUT],
                    start=(k == 0),
                    stop=(k == 7),
                )
            res_sb = ap.tile([1, OUT], f32, tag="res")
            nc.vector.tensor_copy(res_sb[:], pso[:])
            nc.scalar.dma_start(out_d[:], res_sb[:])

    nc.compile()
    return nc


def _prep_inputs(x, s, W_in, W_hh, W_out, b):
    """Shard + fp16-quantize + lay out the inputs for each of the 8 cores."""
    f16 = np.float16
    x_cat = np.concatenate([np.asarray(x), np.asarray(s)]).astype(f16)
    x_cat = np.ascontiguousarray(x_cat.reshape(D, 1))
    Whh16 = np.asarray(W_hh).astype(f16)  # [3, 8192, 8192]
    Win16 = np.ascontiguousarray(np.asarray(W_in).astype(f16))  # [10, 8192]
    Wout16 = np.asarray(W_out).astype(f16)  # [8192, 8]
    b16 = np.asarray(b).astype(f16)  # [5, 8192] (b[4] unused)
    bias0 = np.ascontiguousarray(b16[0].reshape(1, H))

    perms = [_perm_piece(), _perm_ab(), _perm_ab()]  # input layout per layer

    in_maps = []
    for c in range(NCORES):
        cs, ce = c * SH, (c + 1) * SH
        whh_c = np.empty((3, 2, G, 128, GC * HF), f16)
        for li in range(3):
            wcol = Whh16[li][:, cs:ce]  # [8192, 1024]
            wperm = wcol[perms[li]]  # [64, 128, 1024]
            for hf in range(2):
                arr = wperm[:, :, hf * HF : (hf + 1) * HF]  # [64, 128, 512]
                grp = arr.reshape(G, GC, 128, HF).transpose(0, 2, 1, 3)
                whh_c[li, hf] = grp.reshape(G, 128, GC * HF)
        wout_c = np.ascontiguousarray(Wout16[cs:ce, :].reshape(128, 8 * OUT))
        in_maps.append(
            {
                "x_cat": x_cat,
                "w_in": Win16,
                "w_hh": np.ascontiguousarray(whh_c),
                "w_out": wout_c,
                "bias0": bias0,
                "bias": np.ascontiguousarray(b16[1:4, cs:ce].reshape(1, 3 * SH)),
            }
        )
    return in_maps


def kernel(**inputs):
    global LAST_RESULTS
    import os

    from concourse import bass_utils

    if "nc" not in _CACHE:
        _CACHE["nc"] = _build_nc()
    nc = _CACHE["nc"]

    in_maps = _prep_inputs(**inputs)
    trace = bool(int(os.environ.get("BASS_TRACE_KERNEL", "0")))
    res = bass_utils.run_bass_kernel_spmd(
        nc, in_maps, core_ids=list(range(NCORES)), trace=trace
    )
    LAST_RESULTS = res
    partials = np.stack([r["out_partial"][0] for r in res.results])  # [8, 8]
    return partials.sum(axis=0).astype(np.float32)



# revision 3
# speedup vs baseline: 1.2941x; 1.2941x over previous
"""Trainium2 Bass kernel for a 4-layer dense MLP (H=8192), batch=1.

Tensor-parallel over 8 NeuronCores. v2: fp8 weights + 4x column-tiled PE.

  - Hidden-layer weights stream as fp8 e3m4 (x512 scale, de-scaled in the
    sigmoid's scale parameter), halving HBM traffic vs fp16: 8.4 MB per core
    per layer. Activations stay fp16 (mixed-dtype matmul); measured end-to-end
    error vs the f32 reference is ~2e-3 max-rel.

  - Each 512-col half is computed by 4 concurrent column-tiled matmul streams
    (tile_position=(0,32j), 128 cols each) so the PE consumes 4x128 weights
    per cycle; partial rows land on PSUM partitions {0,32,64,96} and a strided
    4-row gather DMA flattens them for the AllGather / output stages.

  - Layer 1 (10x8192) is replicated on every core, computed in 16 passes of
    512 columns bounced through DRAM into the [128, 64] piece-major activation
    layout so layer 2 can contract early pieces while later ones are in
    flight (layer 2's contraction chunks are piece-major via a host-side
    weight-row permutation).

  - Hidden layers 2-4 are column-sharded (core c owns 1024 columns), each
    computed as two 512-col halves A/B. Half A's AllGather fires mid-layer and
    overlaps half B's compute; the next layer's contraction chunks are ordered
    so chunks 0-31 touch only gathered-A data and 32-63 only B.

  - The output layer (8192x8) is row-sharded: each core emits a partial [8]
    which the host sums. A dummy AllGather at kernel start absorbs the
    one-time ncfw rendezvous barrier.

  - Weight DMAs are 1 MiB [128, 8192] fp8 tiles alternating between the two
    HWDGE rings (sync/SP and scalar/ACT) with a 16-buffer (16 MiB, ~2 layers)
    prefetch window, so the weight stream never stalls on a single ring's
    completion latency.
"""

import numpy as np

H = 8192
D = 10  # input layer size (4 + 6)
OUT = 8
NCORES = 8
SH = H // NCORES  # 1024 columns per core
HF = 512  # half-width
CT = 128  # column-tile width (4 tiles per half)
KC = 64  # contraction chunks of 128 rows
GC = 16  # chunks per DMA group (1 MiB per DMA at fp8)
G = KC // GC  # 4 groups per half
WBUFS = 16  # in-flight weight DMA buffers (16 MiB SBUF)
WSCALE = 512.0  # fp8 weight pre-scale (power of 2; e3m4 normal range)

LAST_RESULTS = None
_CACHE = {}


def _perm_piece():
    """Layer-2 input layout: a_sb[p, k] = a1[(k//8)*1024 + p*8 + (k%8)].
    Returns rows[k, p] = global row index feeding chunk k, partition p."""
    k = np.arange(KC)[:, None]
    p = np.arange(128)[None, :]
    return (k // 8) * 1024 + p * 8 + (k % 8)


def _perm_ab():
    """Layer-3/4 input layout: chunks 0-31 hold the gathered A-halves
    (columns [0,512) of every rank), chunks 32-63 the B-halves.
    a_sb[p, k] = half_flat[p*32 + k%32] with half = k//32, and
    half_flat[i] = a_full[(i//512)*1024 + 512*half + i%512]."""
    k = np.arange(KC)[:, None]
    p = np.arange(128)[None, :]
    half = k // 32
    i = p * 32 + (k % 32)
    return (i // 512) * 1024 + 512 * half + (i % 512)


def _build_nc():
    import concourse.bacc as bacc
    import concourse.mybir as mybir
    import concourse.tile as tile

    f16 = mybir.dt.float16
    f32 = mybir.dt.float32
    f8 = mybir.dt.float8e3
    SIG = mybir.ActivationFunctionType.Sigmoid
    RG = [list(range(NCORES))]
    INV = 1.0 / WSCALE

    nc = bacc.Bacc(
        "TRN2", target_bir_lowering=False, debug=False, num_devices=NCORES
    )

    x_d = nc.dram_tensor("x_cat", [D, 1], f16, kind="ExternalInput")
    win_d = nc.dram_tensor("w_in", [D, H], f16, kind="ExternalInput")
    whh_d = nc.dram_tensor("w_hh", [3, 2, G, 128, GC * HF], f8, kind="ExternalInput")
    wout_d = nc.dram_tensor("w_out", [128, 8 * OUT], f16, kind="ExternalInput")
    bias0_d = nc.dram_tensor("bias0", [1, H], f16, kind="ExternalInput")
    bias_d = nc.dram_tensor("bias", [1, 3 * SH], f16, kind="ExternalInput")
    out_d = nc.dram_tensor("out_partial", [1, OUT], f32, kind="ExternalOutput")

    with tile.TileContext(nc) as tc:
        with (
            tc.tile_pool(name="const", bufs=1) as cp,
            tc.tile_pool(name="wpool", bufs=WBUFS) as wp,
            tc.tile_pool(name="apool", bufs=2) as ap,
            tc.tile_pool(name="pspool", bufs=2, space="PSUM") as pp,
            tc.tile_pool(name="dpool", bufs=2, space="DRAM") as dp,
        ):
            one_sb = cp.tile([1, 1], f16)
            nc.gpsimd.memset(one_sb[:], 1.0)

            # Dummy collective: absorbs the one-time ncfw rendezvous barrier
            # concurrently with layer-1 compute + weight prefetch.
            warm_sb = cp.tile([1, 16], f16)
            nc.gpsimd.memset(warm_sb[:], 0.0)
            warm_in = dp.tile([1, 16], f16, tag="warmin")
            warm_out = dp.tile([8, 16], f16, tag="warmout")
            nc.gpsimd.dma_start(warm_in[:], warm_sb[:])
            nc.gpsimd.collective_compute(
                "AllGather",
                mybir.AluOpType.bypass,
                replica_groups=RG,
                ins=[warm_in.opt()],
                outs=[warm_out.opt()],
            )

            x_sb = cp.tile([D, 1], f16)
            nc.scalar.dma_start(x_sb[:], x_d[:])
            win_sb = cp.tile([D, H], f16)
            nc.scalar.dma_start(win_sb[:], win_d[:])
            bias0_sb = cp.tile([1, H], f16)
            nc.scalar.dma_start(bias0_sb[:], bias0_d[:])
            bias_sb = cp.tile([1, 3 * SH], f16)
            nc.scalar.dma_start(bias_sb[:], bias_d[:])
            wout_sb = cp.tile([128, 8 * OUT], f16)
            nc.scalar.dma_start(wout_sb[:], wout_d[:])

            # ---- Layer 1, replicated: 16 passes of 512 cols, piecewise
            # bounce into the [128, 64] piece-major layout for layer 2 ----
            act1_sb = cp.tile([1, H], f16)
            a1_d = dp.tile([16, HF], f16, tag="a1")
            a_sb = ap.tile([128, KC], f16, tag="a")
            for q in range(16):
                h, odd = q // 2, q % 2
                lo = q * HF
                ps1 = pp.tile([1, HF], f32, tag=f"psA{odd}", bufs=1)
                nc.tensor.matmul(
                    ps1[:],
                    x_sb[:],
                    win_sb[:, lo : lo + HF],
                    start=True,
                    stop=False,
                )
                nc.tensor.matmul(
                    ps1[:],
                    one_sb[:],
                    bias0_sb[:, lo : lo + HF],
                    start=False,
                    stop=True,
                )
                nc.scalar.activation(act1_sb[:, lo : lo + HF], ps1[:], SIG)
                nc.scalar.dma_start(
                    a1_d[q : q + 1, :], act1_sb[:, lo : lo + HF]
                )
                nc.scalar.dma_start(
                    a_sb[64 * odd : 64 * odd + 64, 8 * h : 8 * h + 8],
                    a1_d[q].rearrange("(p k) -> p k", p=64),
                )

            # ---- Hidden layers 2-4: two 512-col halves, 4x col-tiled PE,
            # AG-A at mid-layer ----
            act_half = [None, None]
            for li in range(3):
                a_next = (
                    ap.tile([128, KC], f16, tag="a", name="a_next")
                    if li < 2
                    else None
                )
                for hf in range(2):
                    ps = pp.tile([128, CT], f32, tag=f"ps{hf}")
                    for g in range(G):
                        wt = wp.tile([128, GC * HF], f8, tag="w")
                        eng = nc.sync if (g + hf) % 2 == 0 else nc.scalar
                        eng.dma_start(wt[:], whh_d[li, hf, g])
                        for c in range(GC):
                            k = g * GC + c
                            for j in range(4):
                                nc.tensor.matmul(
                                    ps[32 * j : 32 * j + 1, :],
                                    a_sb[:, k : k + 1],
                                    wt[:, c * HF + CT * j : c * HF + CT * (j + 1)],
                                    start=(k == 0),
                                    stop=False,
                                    tile_position=(0, 32 * j),
                                )
                    bo = li * SH + hf * HF
                    for j in range(4):
                        nc.tensor.matmul(
                            ps[32 * j : 32 * j + 1, :],
                            one_sb[:],
                            bias_sb[:, bo + CT * j : bo + CT * (j + 1)],
                            start=False,
                            stop=True,
                            tile_position=(0, 32 * j),
                        )
                    act_h = ap.tile([128, CT], f16, tag=f"act{hf}")
                    for j in range(4):
                        nc.scalar.activation(
                            act_h[32 * j : 32 * j + 1, :],
                            ps[32 * j : 32 * j + 1, :],
                            SIG,
                            scale=INV,
                        )
                    act_half[hf] = act_h
                    if li < 2:
                        cc_in = dp.tile([1, HF], f16, tag=f"ccin{hf}")
                        cc_out = dp.tile([128, 32], f16, tag=f"ccout{hf}")
                        nc.gpsimd.dma_start(cc_in[:], act_h[0:97:32, :])
                        nc.gpsimd.collective_compute(
                            "AllGather",
                            mybir.AluOpType.bypass,
                            replica_groups=RG,
                            ins=[cc_in.opt()],
                            outs=[cc_out.opt()],
                        )
                        nc.gpsimd.dma_start(
                            a_next[:, 32 * hf : 32 * hf + 32], cc_out[:]
                        )
                if li < 2:
                    a_sb = a_next

            # ---- Output layer: row-sharded, partial [8] per core ----
            sc = dp.tile([1, SH], f16, tag="sc")
            nc.gpsimd.dma_start(sc[:, 0:HF], act_half[0][0:97:32, :])
            nc.gpsimd.dma_start(sc[:, HF:SH], act_half[1][0:97:32, :])
            a2_sb = ap.tile([128, 8], f16, tag="a2")
            nc.gpsimd.dma_start(
                a2_sb[:], sc.rearrange("one (p k) -> (one p) k", p=128)
            )
            pso = pp.tile([1, OUT], f32, tag="psO", bufs=1)
            for k in range(8):
                nc.tensor.matmul(
                    pso[:],
                    a2_sb[:, k : k + 1],
                    wout_sb[:, k * OUT : (k + 1) * OUT],
                    start=(k == 0),
                    stop=(k == 7),
                )
            res_sb = ap.tile([1, OUT], f32, tag="res")
            nc.vector.tensor_copy(res_sb[:], pso[:])
            nc.scalar.dma_start(out_d[:], res_sb[:])

    nc.compile()
    return nc


def _prep_inputs(x, s, W_in, W_hh, W_out, b):
    """Shard + quantize + lay out the inputs for each of the 8 cores."""
    import ml_dtypes

    f16 = np.float16
    f8 = ml_dtypes.float8_e3m4
    x_cat = np.concatenate([np.asarray(x), np.asarray(s)]).astype(f16)
    x_cat = np.ascontiguousarray(x_cat.reshape(D, 1))
    Whh8 = (np.asarray(W_hh) * WSCALE).astype(f8)  # [3, 8192, 8192]
    Win16 = np.ascontiguousarray(np.asarray(W_in).astype(f16))  # [10, 8192]
    Wout16 = np.asarray(W_out).astype(f16)  # [8192, 8]
    b_np = np.asarray(b)
    bias0 = np.ascontiguousarray(b_np[0].astype(f16).reshape(1, H))
    bias_sc = (b_np[1:4] * WSCALE).astype(f16)  # [3, 8192], pre-scaled

    perms = [_perm_piece(), _perm_ab(), _perm_ab()]  # input layout per layer

    in_maps = []
    for c in range(NCORES):
        cs, ce = c * SH, (c + 1) * SH
        whh_c = np.empty((3, 2, G, 128, GC * HF), f8)
        for li in range(3):
            wcol = Whh8[li][:, cs:ce]  # [8192, 1024]
            wperm = wcol[perms[li]]  # [64, 128, 1024]
            for hf in range(2):
                arr = wperm[:, :, hf * HF : (hf + 1) * HF]  # [64, 128, 512]
                grp = arr.reshape(G, GC, 128, HF).transpose(0, 2, 1, 3)
                whh_c[li, hf] = grp.reshape(G, 128, GC * HF)
        wout_c = np.ascontiguousarray(Wout16[cs:ce, :].reshape(128, 8 * OUT))
        in_maps.append(
            {
                "x_cat": x_cat,
                "w_in": Win16,
                "w_hh": np.ascontiguousarray(whh_c),
                "w_out": wout_c,
                "bias0": bias0,
                "bias": np.ascontiguousarray(bias_sc[:, cs:ce].reshape(1, 3 * SH)),
            }
        )
    return in_maps


def kernel(**inputs):
    global LAST_RESULTS
    import os

    from concourse import bass_utils

    if "nc" not in _CACHE:
        _CACHE["nc"] = _build_nc()
    nc = _CACHE["nc"]

    in_maps = _prep_inputs(**inputs)
    trace = bool(int(os.environ.get("BASS_TRACE_KERNEL", "0")))
    res = bass_utils.run_bass_kernel_spmd(
        nc, in_maps, core_ids=list(range(NCORES)), trace=trace
    )
    LAST_RESULTS = res
    partials = np.stack([r["out_partial"][0] for r in res.results])  # [8, 8]
    return partials.sum(axis=0).astype(np.float32)


# revision 6
# speedup vs baseline: 1.5105x; 1.1672x over previous
"""Trainium2 Bass kernel for a 4-layer dense MLP (H=8192), batch=1.

Tensor-parallel over 8 NeuronCores. v3: fp8 weights, 4x column-tiled PE,
latency-ordered collective chains.

  - Hidden-layer weights stream as fp8 e3m4 (x512 scale, de-scaled in the
    sigmoid's scale parameter), halving HBM traffic vs fp16: 8.4 MB per core
    per layer. Activations stay fp16 (mixed-dtype matmul); measured end-to-end
    error vs the f32 reference is ~2e-3 max-rel.

  - Each 512-col half is computed by 4 concurrent column-tiled matmul streams
    (tile_position=(0,32j), 128 cols each) so the PE consumes 4x128 weights
    per cycle; partial rows land on PSUM partitions {0,32,64,96}, one
    activation call covers all four, and strided 4-row gather DMAs flatten
    them for the AllGather / output stages.

  - All weight DMAs ride the sync (SP) HWDGE ring exclusively: putting any on
    the scalar ring stalls them behind dependency-waiting activations (in-
    order sequencer). cc_in gathers ride scalar right behind their act;
    AllGathers + post-AG scatters ride gpsimd, with both AGs of a layer
    emitted before the (AG-blocked) a_next writes so neither AG is head-of-
    line blocked.

  - Layer 1 folds its bias into an 11th weight row ([x;s;1] @ [W_in;b0]) and
    runs 4 column-tiled passes of 2048 cols each, bounced through DRAM into
    the [128, 64] piece-major layout; layer 2's group g contracts pass g's
    chunks as soon as they land.

  - Hidden layers 2-4 are column-sharded (core owns 1024 cols), two 512-col
    halves A/B per layer; half A's AllGather overlaps half B's compute; the
    next layer's contraction chunks are ordered A-first (host-side weight-row
    permutation) so it starts when AG-A lands.

  - Output layer (8192x8) is row-sharded: per-half SBUF->SBUF gathers into
    [64,8] tiles, 8+8 accumulating matmuls (A-half fires during L4-B), each
    core emits a partial [8] summed on the host. A dummy AllGather at kernel
    start absorbs the one-time ncfw rendezvous barrier.
"""

import numpy as np

H = 8192
D = 11  # input layer size (4 + 6) + folded bias row
OUT = 8
NCORES = 8
SH = H // NCORES  # 1024 columns per core
HF = 512  # half-width
CT = 128  # column-tile width (4 tiles per half)
KC = 64  # contraction chunks of 128 rows
GC = 16  # chunks per DMA group (1 MiB per DMA at fp8)
G = KC // GC  # 4 groups per half
WBUFS = 20  # in-flight weight DMA buffers (20 MiB SBUF)
WSCALE = 512.0  # fp8 weight pre-scale (power of 2; e3m4 normal range)
L1P = 2048  # layer-1 pass width (4 passes)

LAST_RESULTS = None
_CACHE = {}


def _perm_piece():
    """Layer-2 input layout: a_sb[p, k] = a1[(k//8)*1024 + p*8 + (k%8)].
    Returns rows[k, p] = global row index feeding chunk k, partition p."""
    k = np.arange(KC)[:, None]
    p = np.arange(128)[None, :]
    return (k // 8) * 1024 + p * 8 + (k % 8)


def _perm_ab():
    """Layer-3/4 input layout: chunks 0-31 hold the gathered A-halves
    (columns [0,512) of every rank), chunks 32-63 the B-halves.
    a_sb[p, k] = half_flat[p*32 + k%32] with half = k//32, and
    half_flat[i] = a_full[(i//512)*1024 + 512*half + i%512]."""
    k = np.arange(KC)[:, None]
    p = np.arange(128)[None, :]
    half = k // 32
    i = p * 32 + (k % 32)
    return (i // 512) * 1024 + 512 * half + (i % 512)


def _build_nc():
    import concourse.bacc as bacc
    import concourse.mybir as mybir
    import concourse.tile as tile

    f16 = mybir.dt.float16
    f32 = mybir.dt.float32
    f8 = mybir.dt.float8e3
    SIG = mybir.ActivationFunctionType.Sigmoid
    RG = [list(range(NCORES))]
    INV = 1.0 / WSCALE

    nc = bacc.Bacc(
        "TRN2", target_bir_lowering=False, debug=False, num_devices=NCORES
    )

    x_d = nc.dram_tensor("x_cat", [D, 1], f16, kind="ExternalInput")
    win_d = nc.dram_tensor("w_in", [D, H], f16, kind="ExternalInput")
    whh_d = nc.dram_tensor("w_hh", [3, 2, G, 128, GC * HF], f8, kind="ExternalInput")
    wout_d = nc.dram_tensor("w_out", [128, 8 * OUT], f16, kind="ExternalInput")
    bias_d = nc.dram_tensor("bias", [1, 3 * SH], f16, kind="ExternalInput")
    out_d = nc.dram_tensor("out_partial", [1, OUT], f32, kind="ExternalOutput")

    with tile.TileContext(nc) as tc:
        with (
            tc.tile_pool(name="const", bufs=1) as cp,
            tc.tile_pool(name="wpool", bufs=WBUFS) as wp,
            tc.tile_pool(name="apool", bufs=2) as ap,
            tc.tile_pool(name="pspool", bufs=2, space="PSUM") as pp,
            tc.tile_pool(name="dpool", bufs=2, space="DRAM") as dp,
        ):
            one_sb = cp.tile([1, 1], f16)
            nc.gpsimd.memset(one_sb[:], 1.0)

            # Dummy collective: absorbs the one-time ncfw rendezvous barrier
            # concurrently with layer-1 compute + weight prefetch.
            warm_sb = cp.tile([1, 16], f16)
            nc.gpsimd.memset(warm_sb[:], 0.0)
            warm_in = dp.tile([1, 16], f16, tag="warmin")
            warm_out = dp.tile([8, 16], f16, tag="warmout")
            nc.gpsimd.dma_start(warm_in[:], warm_sb[:])
            nc.gpsimd.collective_compute(
                "AllGather",
                mybir.AluOpType.bypass,
                replica_groups=RG,
                ins=[warm_in.opt()],
                outs=[warm_out.opt()],
            )

            x_sb = cp.tile([D, 1], f16)
            nc.scalar.dma_start(x_sb[:], x_d[:])
            win_sb = cp.tile([D, H], f16)
            nc.scalar.dma_start(win_sb[:], win_d[:])
            bias_sb = cp.tile([1, 3 * SH], f16)
            nc.scalar.dma_start(bias_sb[:], bias_d[:])
            woutA_sb = cp.tile([64, 8 * OUT], f16)
            nc.scalar.dma_start(woutA_sb[:], wout_d[0:64, :])
            woutB_sb = cp.tile([64, 8 * OUT], f16)
            nc.scalar.dma_start(woutB_sb[:], wout_d[64:128, :])

            # ---- Layer 1, replicated: 4 col-tiled passes of 2048 cols,
            # bounced into the [128, 64] piece-major layout for layer 2 ----
            a1_d = dp.tile([4, L1P], f16, tag="a1")
            a_sb = ap.tile([128, KC], f16, tag="a")
            for p in range(4):
                psA = pp.tile([128, HF], f32, tag=f"psA{p % 2}", bufs=1)
                for j in range(4):
                    nc.tensor.matmul(
                        psA[32 * j : 32 * j + 1, :],
                        x_sb[:],
                        win_sb[:, p * L1P + j * HF : p * L1P + (j + 1) * HF],
                        start=True,
                        stop=True,
                        tile_position=(0, 32 * j),
                    )
                act1 = ap.tile([128, HF], f16, tag=f"act1_{p % 2}")
                nc.scalar.activation(act1[0:97, :], psA[0:97, :], SIG)
                nc.scalar.dma_start(a1_d[p], act1[0:97:32, :])
                nc.scalar.dma_start(
                    a_sb[:, 16 * p : 16 * p + 16],
                    a1_d[p].rearrange("(h p m) -> p h m", h=2, p=128),
                )

            # ---- Hidden layers 2-4: two 512-col halves, 4x col-tiled PE,
            # one act per half, AG-A overlapping half B ----
            act_half = [None, None]
            a2x = [None, None]
            cc_out = [None, None]
            pso = None
            for li in range(3):
                a_next = (
                    ap.tile([128, KC], f16, tag="a", name="a_next")
                    if li < 2
                    else None
                )
                for hf in range(2):
                    ps = pp.tile([128, CT], f32, tag=f"ps{hf}")
                    for g in range(G):
                        wt = wp.tile([128, GC * HF], f8, tag="w")
                        nc.sync.dma_start(wt[:], whh_d[li, hf, g])
                        for c in range(GC):
                            k = g * GC + c
                            for j in range(4):
                                nc.tensor.matmul(
                                    ps[32 * j : 32 * j + 1, :],
                                    a_sb[:, k : k + 1],
                                    wt[:, c * HF + CT * j : c * HF + CT * (j + 1)],
                                    start=(k == 0),
                                    stop=False,
                                    tile_position=(0, 32 * j),
                                )
                    bo = li * SH + hf * HF
                    for j in range(4):
                        nc.tensor.matmul(
                            ps[32 * j : 32 * j + 1, :],
                            one_sb[:],
                            bias_sb[:, bo + CT * j : bo + CT * (j + 1)],
                            start=False,
                            stop=True,
                            tile_position=(0, 32 * j),
                        )
                    act_h = ap.tile([128, CT], f16, tag=f"act{hf}")
                    nc.scalar.activation(
                        act_h[0:97, :], ps[0:97, :], SIG, scale=INV
                    )
                    act_half[hf] = act_h
                    if li < 2:
                        cc_in = dp.tile([1, HF], f16, tag=f"ccin{hf}")
                        cc_out[hf] = dp.tile(
                            [128, 32], f16, tag=f"ccout{hf}", name=f"ccout{hf}"
                        )
                        nc.scalar.dma_start(cc_in[:], act_h[0:97:32, :])
                        nc.gpsimd.collective_compute(
                            "AllGather",
                            mybir.AluOpType.bypass,
                            replica_groups=RG,
                            ins=[cc_in.opt()],
                            outs=[cc_out[hf].opt()],
                        )
                    else:
                        # output stage, this half: gather into [64, 8] and
                        # run its 8 accumulating matmuls (A overlaps B's
                        # weight stream)
                        a2 = ap.tile([64, OUT], f16, tag=f"a2{hf}")
                        nc.gpsimd.dma_start(a2[:], act_h[0:97:32, :])
                        a2x[hf] = a2
                        wout_sb = woutA_sb if hf == 0 else woutB_sb
                        if hf == 0:
                            pso = pp.tile([1, OUT], f32, tag="psO", bufs=1)
                        for k in range(8):
                            nc.tensor.matmul(
                                pso[:],
                                a2x[hf][:, k : k + 1],
                                wout_sb[:, k * OUT : (k + 1) * OUT],
                                start=(hf == 0 and k == 0),
                                stop=(hf == 1 and k == 7),
                            )
                # post-AG scatters AFTER both AGs are dispatched, so AG-B is
                # never head-of-line blocked behind AG-A's completion.
                if li < 2:
                    for hf in range(2):
                        nc.gpsimd.dma_start(
                            a_next[:, 32 * hf : 32 * hf + 32], cc_out[hf][:]
                        )
                    a_sb = a_next

            res_sb = ap.tile([1, OUT], f32, tag="res")
            nc.vector.tensor_copy(res_sb[:], pso[:])
            nc.scalar.dma_start(out_d[:], res_sb[:])

    nc.compile()
    return nc


def _prep_inputs(x, s, W_in, W_hh, W_out, b):
    """Shard + quantize + lay out the inputs for each of the 8 cores."""
    import ml_dtypes

    f16 = np.float16
    f8 = ml_dtypes.float8_e3m4
    b_np = np.asarray(b)
    x_cat = np.concatenate(
        [np.asarray(x), np.asarray(s), np.ones(1, np.float32)]
    ).astype(f16)
    x_cat = np.ascontiguousarray(x_cat.reshape(D, 1))
    Whh8 = (np.asarray(W_hh) * WSCALE).astype(f8)  # [3, 8192, 8192]
    Win16 = np.ascontiguousarray(
        np.vstack([np.asarray(W_in), b_np[0:1]]).astype(f16)
    )  # [11, 8192] with bias row folded in
    Wout16 = np.asarray(W_out).astype(f16)  # [8192, 8]
    bias_sc = (b_np[1:4] * WSCALE).astype(f16)  # [3, 8192], pre-scaled

    perms = [_perm_piece(), _perm_ab(), _perm_ab()]  # input layout per layer

    in_maps = []
    for c in range(NCORES):
        cs, ce = c * SH, (c + 1) * SH
        whh_c = np.empty((3, 2, G, 128, GC * HF), f8)
        for li in range(3):
            wcol = Whh8[li][:, cs:ce]  # [8192, 1024]
            wperm = wcol[perms[li]]  # [64, 128, 1024]
            for hf in range(2):
                arr = wperm[:, :, hf * HF : (hf + 1) * HF]  # [64, 128, 512]
                grp = arr.reshape(G, GC, 128, HF).transpose(0, 2, 1, 3)
                whh_c[li, hf] = grp.reshape(G, 128, GC * HF)
        wout_c = np.ascontiguousarray(Wout16[cs:ce, :].reshape(128, 8 * OUT))
        in_maps.append(
            {
                "x_cat": x_cat,
                "w_in": Win16,
                "w_hh": np.ascontiguousarray(whh_c),
                "w_out": wout_c,
                "bias": np.ascontiguousarray(bias_sc[:, cs:ce].reshape(1, 3 * SH)),
            }
        )
    return in_maps


def kernel(**inputs):
    global LAST_RESULTS
    import os

    from concourse import bass_utils

    if "nc" not in _CACHE:
        _CACHE["nc"] = _build_nc()
    nc = _CACHE["nc"]

    in_maps = _prep_inputs(**inputs)
    trace = bool(int(os.environ.get("BASS_TRACE_KERNEL", "0")))
    res = bass_utils.run_bass_kernel_spmd(
        nc, in_maps, core_ids=list(range(NCORES)), trace=trace
    )
    LAST_RESULTS = res
    partials = np.stack([r["out_partial"][0] for r in res.results])  # [8, 8]
    return partials.sum(axis=0).astype(np.float32)
